# revision 1
# baseline (speedup 1.0000x reference)
"""Trainium2 Bass kernel for nn_CustomDecoderLayer (FAVOR+ decoder layer).

Sharding: 8 cores = 4 batch groups x 2-way tensor parallel (heads/ffn),
pair all-reduces after Wo and W2. Per-core program is SPMD-identical.
Activations are F-layout on chip (features on partitions, seq on free dim).
"""
import os
import sys
sys.path.insert(0, "/opt/trn_rl_repo")
from contextlib import ExitStack

import numpy as np
import ml_dtypes

import concourse.bass as bass
import concourse.mybir as mybir
import concourse.tile as tile
from concourse import bacc, bass_isa

f32 = mybir.dt.float32
f32r = mybir.dt.float32r
bf16 = mybir.dt.bfloat16
AF = mybir.ActivationFunctionType
AX = mybir.AxisListType
ALU = mybir.AluOpType

D, H, DH, M = 1024, 16, 64, 256
S, B, F = 2048, 4, 4096
HL, FL = 8, 2048
C2 = 0.5 * (DH ** -0.5)      # 0.0625, exact in bf16
EPS16 = 1.0e-6 * 16.0
KD = D // 128                # 8
NCH = 4                      # row chunks of 512
RT = S // 128                # 16
RG = [[0, 1], [2, 3], [4, 5], [6, 7]]

_CACHE = {}
KSTAGE = {"ln1": 0, "saproj": 0.3, "saphi": 0.6, "sa": 1, "ca": 2, "full": 3}[os.environ.get("KSTAGE", "full")]
KREPS = int(os.environ.get("KREPS", "1"))


def _load_x(nc, tc, ctx, src_ap, add_ap):
    """Load residual (128, KD, S) f32r tile; optionally add bf16 AR result."""
    xp = ctx.enter_context(tc.tile_pool(name="xp", bufs=1))
    sp = ctx.enter_context(tc.tile_pool(name="xsp", bufs=2))
    x_t = xp.tile([128, KD, S], f32r, tag="x_t", name="x_t")
    for kd in range(KD):
        ks = slice(kd * 128, (kd + 1) * 128)
        if add_ap is None:
            nc.sync.dma_start(out=x_t[:, kd, :], in_=src_ap[ks, :].bitcast(f32r))
        else:
            xa = sp.tile([128, S], f32, tag="xa", name="xa")
            nc.sync.dma_start(out=xa[:], in_=src_ap[ks, :])
            ar = sp.tile([128, S], f32, tag="ar", name="ar")
            nc.gpsimd.dma_start(out=ar[:], in_=add_ap[ks, :])
            nc.vector.tensor_add(x_t[:, kd, :], xa[:], ar[:])
    return x_t


def _ln(nc, tc, ctx, x_t, g_t, b_t, out_t, c_invd, c_eps):
    """LayerNorm F-layout: x_t (128, KD, S) f32r -> out_t (128, KD, S)."""
    ps = ctx.enter_context(tc.tile_pool(name="lnps", bufs=2, space="PSUM"))
    sb = ctx.enter_context(tc.tile_pool(name="lnsb", bufs=2))
    for ch in range(NCH):
        cs = bass.ts(ch, 512)
        mv = ps.tile([128, 2, 512], f32, tag="ln_ps", name="ln_ps")
        for kd in range(KD):
            nc.tensor.matmul(mv[:, 0, :], c_invd[:], x_t[:, kd, cs],
                             start=(kd == 0), stop=(kd == KD - 1),
                             skip_group_check=True)
        for kd in range(KD):
            x2 = sb.tile([128, 512], f32r, tag="ln_x2", name="ln_x2")
            nc.vector.tensor_mul(x2[:], x_t[:, kd, cs], x_t[:, kd, cs])
            nc.tensor.matmul(mv[:, 1, :], c_invd[:], x2[:],
                             start=(kd == 0), stop=(kd == KD - 1),
                             skip_group_check=True)
        mu = sb.tile([128, 512], f32, tag="ln_mu", name="ln_mu")
        nc.scalar.copy(mu[:], mv[:, 0, :])
        mu2 = sb.tile([128, 512], f32, tag="ln_mu2", name="ln_mu2")
        nc.vector.tensor_mul(mu2[:], mu[:], mu[:])
        var = sb.tile([128, 512], f32, tag="ln_var", name="ln_var")
        nc.vector.tensor_sub(var[:], mv[:, 1, :], mu2[:])
        sd = sb.tile([128, 512], f32, tag="ln_sd", name="ln_sd")
        nc.scalar.activation(sd[:], var[:], AF.Sqrt, bias=c_eps[:])
        rstd = sb.tile([128, 512], f32, tag="ln_rstd", name="ln_rstd")
        nc.vector.reciprocal(rstd[:], sd[:])
        for kd in range(KD):
            xm = sb.tile([128, 512], f32, tag="ln_xm", name="ln_xm")
            nc.vector.tensor_sub(xm[:], x_t[:, kd, cs], mu[:])
            xn = sb.tile([128, 512], f32, tag="ln_xn", name="ln_xn")
            nc.vector.tensor_mul(xn[:], xm[:], rstd[:])
            nc.vector.tensor_scalar(out_t[:, kd, cs], xn[:],
                                    g_t[:, kd:kd + 1], b_t[:, kd:kd + 1],
                                    ALU.mult, ALU.add)


def _proj_qk(nc, tc, ctx, src_t, w_ap, b_t, out_t, wtag, ps, wp):
    """out_t (128, 4, S) bf16 = (src.T @ W).T ; W DRAM (D, 512) bf16."""
    for mt in range(4):
        for ch in range(NCH):
            cs = bass.ts(ch, 512)
            o_ps = ps.tile([128, 512], f32, tag="proj_ps", name="proj_ps")
            for kd in range(KD):
                wt = wp.tile([128, 128], bf16, tag=wtag, name="w_" + wtag)
                nc.sync.dma_start(
                    out=wt[:], in_=w_ap[kd * 128:(kd + 1) * 128,
                                        mt * 128:(mt + 1) * 128])
                nc.tensor.matmul(o_ps[:], wt[:], src_t[:, kd, cs],
                                 start=(kd == 0), stop=(kd == KD - 1),
                                 skip_group_check=True)
            nc.scalar.activation(out_t[:, mt, cs], o_ps[:], AF.Identity,
                                 bias=b_t[:, mt:mt + 1])


def _proj_v(nc, tc, ctx, src_t, wv_ap, bv_rep, v_aug, ps, wp):
    """v R-layout -> v_aug (128, RT, HL, 65) bf16 (last col preset to 1)."""
    wv_res = ctx.enter_context(tc.tile_pool(name="wvres", bufs=1)) \
        .tile([128, KD, 512], bf16, name="wv_res")
    for kd in range(KD):
        nc.sync.dma_start(out=wv_res[:, kd, :],
                          in_=wv_ap[kd * 128:(kd + 1) * 128, :])
    for rt in range(RT):
        v_ps = ps.tile([128, 512], f32, tag="v_ps", name="v_ps")
        for kd in range(KD):
            nc.tensor.matmul(v_ps[:], src_t[:, kd, rt * 128:(rt + 1) * 128],
                             wv_res[:, kd, :], start=(kd == 0),
                             stop=(kd == KD - 1), skip_group_check=True)
        nc.vector.tensor_add(v_aug[:, rt, :, 0:64], v_ps[:], bv_rep[:])


def _phi_attn(nc, tc, ctx, qT, kT, v_aug, wf_pair, c_negblk, ident_bf, attn_t):
    """FAVOR+ core: per head-pair phi(q), phi(k), kv, out/z -> attn_t."""
    ps_pj = ctx.enter_context(tc.tile_pool(name="ps_pj", bufs=2, space="PSUM"))
    ps_ns = ctx.enter_context(tc.tile_pool(name="ps_ns", bufs=1, space="PSUM"))
    ps_kv = ctx.enter_context(tc.tile_pool(name="ps_kv", bufs=2, space="PSUM"))
    ps_tp = ctx.enter_context(tc.tile_pool(name="ps_tp", bufs=1, space="PSUM"))
    ps_o = ctx.enter_context(tc.tile_pool(name="ps_o", bufs=1, space="PSUM"))
    ps_z = ctx.enter_context(tc.tile_pool(name="ps_z", bufs=1, space="PSUM"))
    sb = ctx.enter_context(tc.tile_pool(name="phisb", bufs=3))
    big = ctx.enter_context(tc.tile_pool(name="phibig", bufs=1))

    for p in range(4):
        E_t = big.tile([128, RT, 2, 257], bf16, tag="E_t", name="E_t")
        nc.vector.memset(E_t[:, :, :, 256:257], EPS16)
        stab_run = sb.tile([128, 2], f32, tag="stab_run", name="stab_run")
        for rt in range(RT):
            rs = bass.ts(rt, 128)
            k2 = sb.tile([128, 128], bf16, tag="k2", name="k2")
            nc.vector.tensor_mul(k2[:], kT[:, p, rs], kT[:, p, rs])
            pj = ps_pj.tile([128, 2, 256], f32, tag="pj", name="pj_k")
            for h in range(2):
                nc.tensor.matmul(pj[:, h, :], kT[:, p, rs], wf_pair[h][:])
            nsq = ps_ns.tile([128, 2], f32, tag="nsq", name="nsq_k")
            nc.tensor.matmul(nsq[:], k2[:], c_negblk[:])
            rmax = sb.tile([128, 2], f32, tag="rmax", name="rmax_k")
            nc.vector.reduce_max(rmax[:], pj[:], axis=AX.X)
            if rt == 0:
                nc.vector.tensor_copy(stab_run[:], rmax[:])
            else:
                nc.vector.tensor_max(stab_run[:], stab_run[:], rmax[:])
            nsq_sb = sb.tile([128, 2], f32, tag="nsq_sb", name="nsq_sb")
            nc.vector.tensor_copy(nsq_sb[:], nsq[:])
            for h in range(2):
                nc.scalar.activation(E_t[:, rt, h, 0:256], pj[:, h, :],
                                     AF.Exp, bias=nsq_sb[:, h:h + 1])
        stab_rep = sb.tile([128, 2], f32, tag="stab_rep", name="stab_rep")
        nc.gpsimd.partition_all_reduce(stab_rep[:], stab_run[:], channels=128,
                                       reduce_op=bass_isa.ReduceOp.max)
        s_t = sb.tile([128, 2], f32, tag="s_t", name="s_t")
        nc.scalar.activation(s_t[:], stab_rep[:], AF.Exp, scale=-1.0)

        kvT_t = sb.tile([128, 2, 2, 65], bf16, tag="kvT_t", name="kvT_t")
        for h in range(2):
            kv_ps = ps_kv.tile([65, 257], f32, tag="kv_ps", name="kv_ps")
            for rt in range(RT):
                nc.tensor.matmul(kv_ps[:], v_aug[:, rt, p * 2 + h, :],
                                 E_t[:, rt, h, :], start=(rt == 0),
                                 stop=(rt == RT - 1), skip_group_check=True)
            csum = sb.tile([65, 1], f32, tag="csum", name="csum")
            nc.vector.tensor_copy(csum[:], kv_ps[:, 256:257])
            kva = sb.tile([65, 256], bf16, tag="kva", name="kva")
            nc.vector.tensor_scalar(kva[:], kv_ps[:, 0:256],
                                    s_t[0:65, h:h + 1], csum[:],
                                    ALU.mult, ALU.add)
            for mt in range(2):
                tp = ps_tp.tile([128, 65], bf16, tag="tp_kv", name="tp_kv")
                nc.tensor.transpose(tp[:], kva[0:65, mt * 128:(mt + 1) * 128],
                                    ident_bf[0:65, 0:65])
                nc.vector.tensor_copy(kvT_t[:, h, mt, :], tp[:])

        pqT = big.tile([128, 2, 2, S], bf16, tag="pqT", name="pqT")
        for rt in range(RT):
            rs = bass.ts(rt, 128)
            q2 = sb.tile([128, 128], bf16, tag="k2", name="q2")
            nc.vector.tensor_mul(q2[:], qT[:, p, rs], qT[:, p, rs])
            pj = ps_pj.tile([128, 2, 256], f32, tag="pj", name="pj_q")
            for h in range(2):
                nc.tensor.matmul(pj[:, h, :], qT[:, p, rs], wf_pair[h][:])
            nsq = ps_ns.tile([128, 2], f32, tag="nsq", name="nsq_q")
            nc.tensor.matmul(nsq[:], q2[:], c_negblk[:])
            rmax = sb.tile([128, 2], f32, tag="rmax", name="rmax_q")
            nc.vector.reduce_max(rmax[:], pj[:], axis=AX.X)
            bias_q = sb.tile([128, 2], f32, tag="bias_q", name="bias_q")
            nc.vector.tensor_sub(bias_q[:], nsq[:], rmax[:])
            pqR = sb.tile([128, 2, 256], bf16, tag="pqR", name="pqR")
            for h in range(2):
                nc.scalar.activation(pqR[:, h, :], pj[:, h, :], AF.Exp,
                                     bias=bias_q[:, h:h + 1])
            for h in range(2):
                for mt in range(2):
                    tp = ps_tp.tile([128, 128], bf16, tag="tp_kv",
                                    name="tp_pq")
                    nc.tensor.transpose(tp[:],
                                        pqR[:, h, mt * 128:(mt + 1) * 128],
                                        ident_bf[:])
                    nc.vector.tensor_scalar_add(pqT[:, h, mt, rs], tp[:],
                                                EPS16)
        for ch in range(NCH):
            cs = bass.ts(ch, 512)
            o_ps = ps_o.tile([128, 512], f32, tag="o_ps", name="o_ps")
            for h in range(2):
                hp = slice(64 * h, 64 * h + 64)
                for mt in range(2):
                    nc.tensor.matmul(o_ps[hp, :], kvT_t[:, h, mt, 0:64],
                                     pqT[:, h, mt, cs], start=(mt == 0),
                                     stop=(mt == 1), skip_group_check=True)
            for h in range(2):
                hp = slice(64 * h, 64 * h + 64)
                z_ps = ps_z.tile([1, 512], f32, tag="z_ps", name="z_ps")
                for mt in range(2):
                    nc.tensor.matmul(z_ps[:], kvT_t[:, h, mt, 64:65],
                                     pqT[:, h, mt, cs], start=(mt == 0),
                                     stop=(mt == 1), skip_group_check=True)
                zr = sb.tile([1, 512], f32, tag="zr", name="zr")
                nc.vector.reciprocal(zr[:], z_ps[:])
                zb = sb.tile([128, 512], f32, tag="zb", name="zb")
                nc.gpsimd.partition_broadcast(zb[:], zr[:], channels=128)
                nc.vector.tensor_mul(attn_t[hp, p, cs], o_ps[hp, :],
                                     zb[hp, :])


def _wo(nc, tc, ctx, attn_t, wo_ap, bo_t, cc_in, cc_out):
    ps = ctx.enter_context(tc.tile_pool(name="wops", bufs=4, space="PSUM"))
    sb = ctx.enter_context(tc.tile_pool(name="wosb", bufs=3))
    wp = ctx.enter_context(tc.tile_pool(name="wowp", bufs=4))
    for md in range(KD):
        for ch in range(NCH):
            cs = bass.ts(ch, 512)
            o_ps = ps.tile([128, 512], f32, tag="wo_ps", name="wo_ps")
            for k4 in range(4):
                wt = wp.tile([128, 128], f32r, tag="wo_w", name="wo_w")
                nc.sync.dma_start(
                    out=wt[:], in_=wo_ap[k4 * 128:(k4 + 1) * 128,
                                         md * 128:(md + 1) * 128]
                    .bitcast(f32r))
                nc.tensor.matmul(o_ps[:], wt[:], attn_t[:, k4, cs],
                                 start=(k4 == 0), stop=(k4 == 3),
                                 skip_group_check=True)
            ev = sb.tile([128, 512], bf16, tag="wo_ev", name="wo_ev")
            nc.scalar.activation(ev[:], o_ps[:], AF.Identity,
                                 bias=bo_t[:, md:md + 1])
            nc.sync.dma_start(out=cc_in[md * 128:(md + 1) * 128, cs],
                              in_=ev[:])
    nc.gpsimd.collective_compute("AllReduce", ALU.add, replica_groups=RG,
                                 ins=[cc_in.opt()], outs=[cc_out.opt()])


def build_nc():
    nc = bacc.Bacc("TRN2", target_bir_lowering=False, debug=False,
                   num_devices=8)

    def din(name, shape, dtype=f32):
        return nc.dram_tensor(name, list(shape), dtype,
                              kind="ExternalInput").ap()

    xT = din("xT", (D, S))
    memT = din("memT", (D, S), bf16)
    wg = {}
    for pre in ("sa", "ca"):
        wg[pre + "_wq"] = din(pre + "_wq", (D, 512), bf16)
        wg[pre + "_wk"] = din(pre + "_wk", (D, 512), bf16)
        wg[pre + "_wv"] = din(pre + "_wv", (D, 512), bf16)
        wg[pre + "_wo"] = din(pre + "_wo", (512, D))
        wg[pre + "_bq"] = din(pre + "_bq", (128, 4))
        wg[pre + "_bk"] = din(pre + "_bk", (128, 4))
        wg[pre + "_bv"] = din(pre + "_bv", (1, 512))
        wg[pre + "_bo"] = din(pre + "_bo", (128, 8))
        wg[pre + "_wf"] = din(pre + "_wf", (64, 256), bf16)
    w1 = din("w1", (D, FL), bf16)
    w2 = din("w2", (FL, D), bf16)
    b1_d = din("b1", (128, 16))
    b2_d = din("b2", (128, 8))
    ln_d = {}
    for i in ("1", "2", "3"):
        ln_d["g" + i] = din("ln%s_g" % i, (128, 8))
        ln_d["b" + i] = din("ln%s_b" % i, (128, 8))
    c_invd_d = din("c_invd", (128, 128))
    c_negblk_d = din("c_negblk", (128, 2), bf16)
    ident_d = din("c_ident", (128, 128), bf16)

    outT = nc.dram_tensor("outT", [D, S], f32, kind="ExternalOutput").ap()

    with tile.TileContext(nc) as tc:
        with ExitStack() as top:
            dram = top.enter_context(tc.tile_pool(name="dram", bufs=1,
                                                  space="DRAM"))
            xcur = dram.tile([D, S], f32, name="xcur")
            ccs = {}
            for i in ("1", "2", "3"):
                ccs["in" + i] = dram.tile([D, S], bf16, name="cc_in" + i)
                ccs["out" + i] = dram.tile([D, S], bf16, name="cc_out" + i)

            const = top.enter_context(tc.tile_pool(name="const", bufs=1))
            c_invd = const.tile([128, 128], f32r, name="c_invd")
            nc.sync.dma_start(out=c_invd[:], in_=c_invd_d[:].bitcast(f32r))
            c_negblk = const.tile([128, 2], bf16, name="c_negblk")
            nc.sync.dma_start(out=c_negblk[:], in_=c_negblk_d[:])
            ident_bf = const.tile([128, 128], bf16, name="ident_bf")
            nc.sync.dma_start(out=ident_bf[:], in_=ident_d[:])
            c_eps = const.tile([128, 1], f32, name="c_eps")
            nc.vector.memset(c_eps[:], 1.0e-5)
            # biases / ln params to SBUF
            cb = {}
            for pre in ("sa", "ca"):
                for nm in ("bq", "bk", "bo"):
                    t = const.tile(list(wg[pre + "_" + nm].shape), f32,
                                   name=pre + nm)
                    nc.sync.dma_start(out=t[:], in_=wg[pre + "_" + nm][:])
                    cb[pre + "_" + nm] = t
                t = const.tile([1, 512], f32, name=pre + "bv")
                nc.sync.dma_start(out=t[:], in_=wg[pre + "_bv"][:])
                cb[pre + "_bv"] = t
                wfl = const.tile([128, 256], bf16, name=pre + "wfl")
                nc.vector.memset(wfl[64:128, :], 0.0)
                nc.sync.dma_start(out=wfl[0:64, :], in_=wg[pre + "_wf"][:])
                wfh = const.tile([128, 256], bf16, name=pre + "wfh")
                nc.vector.memset(wfh[0:64, :], 0.0)
                nc.sync.dma_start(out=wfh[64:128, :], in_=wg[pre + "_wf"][:])
                cb[pre + "_wf"] = (wfl, wfh)
            for nm, d_ap in (("b1", b1_d), ("b2", b2_d)):
                t = const.tile(list(d_ap.shape), f32, name=nm)
                nc.sync.dma_start(out=t[:], in_=d_ap[:])
                cb[nm] = t
            for k, d_ap in ln_d.items():
                t = const.tile([128, 8], f32, name="ln" + k)
                nc.sync.dma_start(out=t[:], in_=d_ap[:])
                cb["ln" + k] = t

            def favor_block(pre, t2_pool_stack, t2_t, kv_from_mem, cc_i, cc_o):
                """Projections + phi + Wo for one attention block."""
                with ExitStack() as fav:
                    res = fav.enter_context(
                        tc.tile_pool(name=pre + "res", bufs=1))
                    qT = res.tile([128, 4, S], bf16, name="qT")
                    kT = res.tile([128, 4, S], bf16, name="kT")
                    v_aug = res.tile([128, RT, HL, 65], bf16, name="v_aug")
                    nc.vector.memset(v_aug[:, :, :, 64:65], 1.0)
                    bv_rep = res.tile([128, 512], f32, name="bv_rep")
                    nc.gpsimd.partition_broadcast(bv_rep[:],
                                                  cb[pre + "_bv"][:],
                                                  channels=128)
                    attn_t = res.tile([128, 4, S], f32r, name="attn_t")
                    with ExitStack() as prj:
                        ps = prj.enter_context(
                            tc.tile_pool(name="prjps", bufs=3, space="PSUM"))
                        wp = prj.enter_context(
                            tc.tile_pool(name="prjwp", bufs=4))
                        _proj_qk(nc, tc, prj, t2_t, wg[pre + "_wq"],
                                 cb[pre + "_bq"], qT, "wq", ps, wp)
                        if kv_from_mem:
                            # free t2 before loading memory tiles
                            t2_pool_stack.close()
                            mem_p = prj.enter_context(
                                tc.tile_pool(name="memp", bufs=1))
                            src = mem_p.tile([128, KD, S], bf16, name="mem_t")
                            for kd in range(KD):
                                nc.sync.dma_start(
                                    out=src[:, kd, :],
                                    in_=memT[kd * 128:(kd + 1) * 128, :])
                        else:
                            src = t2_t
                        _proj_qk(nc, tc, prj, src, wg[pre + "_wk"],
                                 cb[pre + "_bk"], kT, "wk", ps, wp)
                        _proj_v(nc, tc, prj, src, wg[pre + "_wv"], bv_rep,
                                v_aug, ps, wp)
                        if not kv_from_mem:
                            t2_pool_stack.close()
                    if pre == "sa" and KSTAGE == 0.3:
                        with ExitStack() as ph2:
                            sbd = ph2.enter_context(
                                tc.tile_pool(name="dmp3", bufs=2))
                            for i in range(4):
                                dt_ = sbd.tile([128, S], f32, tag="d3",
                                               name="d3")
                                nc.vector.tensor_copy(dt_[:], qT[:, i, :])
                                nc.sync.dma_start(
                                    out=outT[i * 128:(i + 1) * 128, :],
                                    in_=dt_[:])
                                dt2 = sbd.tile([128, S], f32, tag="d3",
                                               name="d3b")
                                nc.vector.tensor_copy(dt2[:], kT[:, i, :])
                                nc.sync.dma_start(
                                    out=outT[(4 + i) * 128:(5 + i) * 128, :],
                                    in_=dt2[:])
                        return
                    with ExitStack() as phc:
                        _phi_attn(nc, tc, phc, qT, kT, v_aug,
                                  cb[pre + "_wf"], c_negblk, ident_bf, attn_t)
                    if pre == "sa" and KSTAGE == 0.6:
                        with ExitStack() as ph2:
                            sbd = ph2.enter_context(
                                tc.tile_pool(name="dmp4", bufs=2))
                            for i in range(4):
                                dt_ = sbd.tile([128, S], f32, tag="d4",
                                               name="d4")
                                nc.vector.tensor_copy(dt_[:], attn_t[:, i, :])
                                nc.sync.dma_start(
                                    out=outT[i * 128:(i + 1) * 128, :],
                                    in_=dt_[:])
                        return
                    with ExitStack() as woc:
                        _wo(nc, tc, woc, attn_t, wg[pre + "_wo"],
                            cb[pre + "_bo"], cc_i, cc_o)

            for _rep in range(KREPS):
                # ---------- P0: LN1(x0) -> t2 ; SA
                t2s = ExitStack()
                t2_t = t2s.enter_context(
                    tc.tile_pool(name="t2p1", bufs=1, side="right")).tile(
                        [128, KD, S], bf16, name="t2_t1")
                with ExitStack() as ph:
                    x_t = _load_x(nc, tc, ph, xT, None)
                    _ln(nc, tc, ph, x_t, cb["lng1"], cb["lnb1"], t2_t,
                        c_invd, c_eps)
                if KSTAGE == 0:
                    with ExitStack() as ph:
                        sbd = ph.enter_context(tc.tile_pool(name="dmp", bufs=2))
                        for kd in range(KD):
                            dt_ = sbd.tile([128, S], f32, tag="dmp", name="dmp")
                            nc.vector.tensor_copy(dt_[:], t2_t[:, kd, :])
                            nc.sync.dma_start(
                                out=outT[kd * 128:(kd + 1) * 128, :], in_=dt_[:])
                    t2s.close()
                    nc_done = True
                else:
                    favor_block("sa", t2s, t2_t, False, ccs["in1"], ccs["out1"])

                def dump_cc(cc):
                    with ExitStack() as ph:
                        sbd = ph.enter_context(tc.tile_pool(name="dmp2", bufs=2))
                        for kd in range(KD):
                            dt_ = sbd.tile([128, S], f32, tag="dmp2", name="dmp2")
                            nc.gpsimd.dma_start(
                                out=dt_[:], in_=cc[kd * 128:(kd + 1) * 128, :])
                            nc.sync.dma_start(
                                out=outT[kd * 128:(kd + 1) * 128, :], in_=dt_[:])

                if KSTAGE == 1:
                    dump_cc(ccs["out1"])
                if KSTAGE >= 2:
                    # ---------- P4: x1 = x0 + AR1 ; LN2 -> t2 ; CA
                    t2s = ExitStack()
                    t2_t = t2s.enter_context(
                        tc.tile_pool(name="t2p2", bufs=1, side="right")).tile(
                            [128, KD, S], bf16, name="t2_t2")
                    with ExitStack() as ph:
                        x_t = _load_x(nc, tc, ph, xT, ccs["out1"])
                        for kd in range(KD):
                            nc.sync.dma_start(out=xcur[kd * 128:(kd + 1) * 128, :],
                                              in_=x_t[:, kd, :].bitcast(f32))
                        _ln(nc, tc, ph, x_t, cb["lng2"], cb["lnb2"], t2_t,
                            c_invd, c_eps)
                    favor_block("ca", t2s, t2_t, True, ccs["in2"], ccs["out2"])
                    if KSTAGE == 2:
                        dump_cc(ccs["out2"])

                if KSTAGE >= 3:
                    # ---------- P8: x2 = x1 + AR2 ; LN3 -> t2b ; FFN
                    t2s = ExitStack()
                    t2b = t2s.enter_context(
                        tc.tile_pool(name="t2p3", bufs=1, side="right")).tile(
                            [128, KD, S], bf16, name="t2b")
                    with ExitStack() as ph:
                        x_t = _load_x(nc, tc, ph, xcur, ccs["out2"])
                        for kd in range(KD):
                            nc.sync.dma_start(out=xcur[kd * 128:(kd + 1) * 128, :],
                                              in_=x_t[:, kd, :].bitcast(f32))
                        _ln(nc, tc, ph, x_t, cb["lng3"], cb["lnb3"], t2b,
                            c_invd, c_eps)
                    with ExitStack() as ph:
                        ps1 = ph.enter_context(tc.tile_pool(name="f1ps", bufs=3,
                                                            space="PSUM"))
                        ps2 = ph.enter_context(tc.tile_pool(name="f2ps", bufs=3,
                                                            space="PSUM"))
                        sb = ph.enter_context(tc.tile_pool(name="ffsb", bufs=3))
                        wp = ph.enter_context(tc.tile_pool(name="ffwp", bufs=4))
                        h1 = ph.enter_context(tc.tile_pool(name="h1p", bufs=1)) \
                            .tile([128, 16, 1024], bf16, name="h1")
                        for rh in range(2):
                            for mf in range(16):
                                for ch in range(2):
                                    cs = bass.ds(rh * 1024 + ch * 512, 512)
                                    o_ps = ps1.tile([128, 512], f32, tag="f1",
                                                    name="f1_ps")
                                    for kd in range(KD):
                                        wt = wp.tile([128, 128], bf16, tag="w1t",
                                                     name="w1t")
                                        nc.sync.dma_start(
                                            out=wt[:],
                                            in_=w1[kd * 128:(kd + 1) * 128,
                                                   mf * 128:(mf + 1) * 128])
                                        nc.tensor.matmul(o_ps[:], wt[:],
                                                         t2b[:, kd, cs],
                                                         start=(kd == 0),
                                                         stop=(kd == KD - 1),
                                                         skip_group_check=True)
                                    nc.scalar.activation(
                                        h1[:, mf, ch * 512:(ch + 1) * 512], o_ps[:],
                                        AF.Relu, bias=cb["b1"][:, mf:mf + 1])
                            for md in range(KD):
                                for ch in range(2):
                                    o_ps = ps2.tile([128, 512], f32, tag="f2",
                                                    name="f2_ps")
                                    for kf in range(16):
                                        wt = wp.tile([128, 128], bf16, tag="w2t",
                                                     name="w2t")
                                        nc.sync.dma_start(
                                            out=wt[:],
                                            in_=w2[kf * 128:(kf + 1) * 128,
                                                   md * 128:(md + 1) * 128])
                                        nc.tensor.matmul(
                                            o_ps[:], wt[:],
                                            h1[:, kf, ch * 512:(ch + 1) * 512],
                                            start=(kf == 0), stop=(kf == 15),
                                            skip_group_check=True)
                                    ev = sb.tile([128, 512], bf16, tag="f2e",
                                                 name="f2_ev")
                                    nc.scalar.activation(ev[:], o_ps[:], AF.Identity,
                                                         bias=cb["b2"][:, md:md + 1])
                                    nc.sync.dma_start(
                                        out=ccs["in3"][md * 128:(md + 1) * 128,
                                                       bass.ds(rh * 1024 + ch * 512,
                                                               512)],
                                        in_=ev[:])
                        nc.gpsimd.collective_compute(
                            "AllReduce", ALU.add, replica_groups=RG,
                            ins=[ccs["in3"].opt()], outs=[ccs["out3"].opt()])
                    t2s.close()

                    # ---------- P10: out = x2 + AR3
                    with ExitStack() as ph:
                        sb = ph.enter_context(tc.tile_pool(name="p10", bufs=3))
                        for kd in range(KD):
                            ks = slice(kd * 128, (kd + 1) * 128)
                            xa = sb.tile([128, S], f32, tag="xa", name="xa10")
                            nc.sync.dma_start(out=xa[:], in_=xcur[ks, :])
                            ar = sb.tile([128, S], f32, tag="ar", name="ar10")
                            nc.gpsimd.dma_start(out=ar[:], in_=ccs["out3"][ks, :])
                            xo = sb.tile([128, S], f32, tag="xo", name="xo10")
                            nc.vector.tensor_add(xo[:], xa[:], ar[:])
                            nc.sync.dma_start(out=outT[ks, :], in_=xo[:])
    nc.finalize()
    return nc


# ------------------------------------------------------------------ host

def _prep_inputs(inputs):
    Cs = DH ** -0.25
    f = np.float32
    bf = ml_dtypes.bfloat16
    inp = {k: np.asarray(v, dtype=f) for k, v in inputs.items()}

    def fshape(vec):
        n = vec.shape[0] // 128
        return np.ascontiguousarray(vec.reshape(n, 128).T)

    consts = {}
    consts["c_invd"] = np.full((128, 128), 1.0 / D, f)
    blk = np.zeros((128, 2), f)
    blk[0:64, 0] = -C2
    blk[64:128, 1] = -C2
    consts["c_negblk"] = blk.astype(bf)
    consts["c_ident"] = np.eye(128, dtype=bf)

    in_maps = []
    for core in range(8):
        b, half = core // 2, core % 2
        hs = slice(half * 512, (half + 1) * 512)
        fs = slice(half * FL, (half + 1) * FL)
        m = dict(consts)
        m["xT"] = np.ascontiguousarray(inp["tgt"][:, b, :].T)
        m["memT"] = np.ascontiguousarray(inp["memory"][:, b, :].T).astype(bf)
        for pre in ("sa", "ca"):
            m[pre + "_wq"] = np.ascontiguousarray(
                inp[pre + "_wq"].T[:, hs]).astype(bf)
            m[pre + "_wk"] = np.ascontiguousarray(
                inp[pre + "_wk"].T[:, hs]).astype(bf)
            m[pre + "_wv"] = np.ascontiguousarray(
                inp[pre + "_wv"].T[:, hs]).astype(bf)
            m[pre + "_wo"] = np.ascontiguousarray(inp[pre + "_wo"].T[hs, :])
            m[pre + "_bq"] = fshape(inp[pre + "_bq"][hs])
            m[pre + "_bk"] = fshape(inp[pre + "_bk"][hs])
            m[pre + "_bv"] = inp[pre + "_bv"][hs].reshape(1, 512).copy()
            m[pre + "_bo"] = fshape(inp[pre + "_bo"] * 0.5)
            m[pre + "_wf"] = np.ascontiguousarray(
                (Cs * inp[pre + "_feat"]).T).astype(bf)
        m["w1"] = np.ascontiguousarray(inp["ff_w1"].T[:, fs]).astype(bf)
        m["w2"] = np.ascontiguousarray(inp["ff_w2"].T[fs, :]).astype(bf)
        m["b1"] = fshape(inp["ff_b1"][fs])
        m["b2"] = fshape(inp["ff_b2"] * 0.5)
        for i in ("1", "2", "3"):
            m["ln%s_g" % i] = fshape(inp["ln%s_g" % i])
            m["ln%s_b" % i] = fshape(inp["ln%s_b" % i])
        in_maps.append(m)
    return in_maps


def _build_exec(nc, n_cores=8):
    import jax
    import jax.numpy as jnp
    from jax.sharding import Mesh, PartitionSpec
    from jax.experimental.shard_map import shard_map
    from concourse import bass2jax as b2j

    b2j.install_neuronx_cc_hook()
    partition_name = (nc.partition_id_tensor.name
                      if nc.partition_id_tensor else None)
    in_names, out_names, out_avals = [], [], []
    for alloc in nc.m.functions[0].allocations:
        if not isinstance(alloc, mybir.MemoryLocationSet):
            continue
        name = alloc.memorylocations[0].name
        if alloc.kind == "ExternalInput":
            if name != partition_name:
                in_names.append(name)
        elif alloc.kind == "ExternalOutput":
            out_names.append(name)
            out_avals.append(jax.core.ShapedArray(
                tuple(alloc.tensor_shape), mybir.dt.np(alloc.dtype)))
    n_params = len(in_names)
    all_in = list(in_names) + list(out_names)
    if partition_name is not None:
        all_in.append(partition_name)

    def _body(*args):
        operands = list(args)
        if partition_name is not None:
            operands.append(b2j.partition_id_tensor())
        outs = b2j._bass_exec_p.bind(
            *operands, out_avals=tuple(out_avals), in_names=tuple(all_in),
            out_names=tuple(out_names), lowering_input_output_aliases=(),
            sim_require_finite=True, sim_require_nnan=True, nc=nc)
        return tuple(outs)

    devices = jax.devices()[:n_cores]
    mesh = Mesh(np.asarray(devices), ("core",))
    n_outs = len(out_names)
    specs = (PartitionSpec("core"),) * (n_params + n_outs)
    out_specs = (PartitionSpec("core"),) * n_outs
    donate = tuple(range(n_params, n_params + n_outs))
    sharded = jax.jit(shard_map(_body, mesh=mesh, in_specs=specs,
                                out_specs=out_specs, check_rep=False),
                      donate_argnums=donate, keep_unused=True)

    def run(in_maps, fetch=True):
        import jax as _jax
        concat = [np.concatenate([np.asarray(in_maps[c][nm])
                                  for c in range(n_cores)], axis=0)
                  for nm in in_names]
        zeros = [np.zeros((n_cores * av.shape[0], *av.shape[1:]), av.dtype)
                 for av in out_avals]
        outs = sharded(*concat, *zeros)
        if not fetch:
            _jax.block_until_ready(outs)
            return None
        return [{nm: np.asarray(outs[i]).reshape(
            n_cores, *out_avals[i].shape)[c]
            for i, nm in enumerate(out_names)} for c in range(n_cores)]

    def time_exec(in_maps, iters=8):
        """Wall-time the sharded exec with device-resident inputs."""
        import time as _time
        import jax as _jax
        from jax.sharding import NamedSharding
        sh = NamedSharding(mesh, PartitionSpec("core"))
        concat = [np.concatenate([np.asarray(in_maps[c][nm])
                                  for c in range(n_cores)], axis=0)
                  for nm in in_names]
        dev_in = _jax.device_put(concat, [sh] * len(concat))
        _jax.block_until_ready(dev_in)
        zeros = [np.zeros((n_cores * av.shape[0], *av.shape[1:]), av.dtype)
                 for av in out_avals]
        times = []
        for _ in range(iters):
            zd = _jax.device_put(zeros, [sh] * len(zeros))
            _jax.block_until_ready(zd)
            t0 = _time.time()
            outs = sharded(*dev_in, *zd)
            _jax.block_until_ready(outs)
            times.append(_time.time() - t0)
        return times

    run.in_names = in_names
    run.time_exec = time_exec
    return run


def _get_exec():
    if "exec" not in _CACHE:
        nc = build_nc()
        _CACHE["exec"] = _build_exec(nc)
    return _CACHE["exec"]


def kernel(**inputs):
    run = _get_exec()
    in_maps = _prep_inputs(inputs)
    res = run(in_maps)
    out = np.empty((S, B, D), np.float32)
    for b in range(B):
        out[:, b, :] = res[2 * b]["outT"].T
    return out



# revision 23
# speedup vs baseline: 1.1159x; 1.1159x over previous
"""Trainium2 Bass kernel for nn_CustomDecoderLayer (FAVOR+ decoder layer).

Sharding: 8 cores = 4 batch x 2 sequence-halves. Each core computes the
full layer (all 16 heads, full FFN) on its 1024 local positions; the only
cross-core data is the linear-attention summary (kv, ksum, vsum) and the
k-stabilizer max -- two tiny pair AllReduces per attention, overlapped
with compute. Host assembles the two sequence halves; no output collective.

FAVOR+ math (exact up to the q-stab proxy, validated 6e-4 rel err):
  phi_k scaled by sqrt(M)*e^{stab_k}:  Ek = exp(projk - sq_k); kv/ksum/vsum
  accumulated via an appended ones-column; after the AllReduce the eps term
  adds beta*vsum with beta = eps*sqrt(M)*e^{stab_k} (stab_k via AR-max).
  phi_q scaled per-row by sqrt(M)*e^{sq+stab_q}: Eq = exp(projq) and the
  eps term becomes a rank-1 row gamma[s]*colkv[d] with
  gamma = eps*sqrt(M)*e^{sq}*rowsum  (log-sum-exp proxy for stab_q),
  applied as a K=1 matmul into the same PSUM accumulation.
"""
import math
import os
import sys
sys.path.insert(0, "/opt/trn_rl_repo")
from contextlib import ExitStack

import numpy as np
import ml_dtypes

import concourse.bass as bass
import concourse.mybir as mybir
import concourse.tile as tile
from concourse import bacc, bass_isa

f32 = mybir.dt.float32
bf16 = mybir.dt.bfloat16
AF = mybir.ActivationFunctionType
AX = mybir.AxisListType
ALU = mybir.AluOpType

D, H, DH, M = 1024, 16, 64, 256
S, B, F = 2048, 4, 4096
S2 = 1024                      # local sequence per core
KD = D // 128                  # 8 feature tiles
RT = S2 // 128                 # 8 row tiles
NCH = S2 // 512                # 2 column chunks
NP = H // 2                    # 8 head pairs
MF = F // 128                  # 32
LNA = math.log(1e-6 * math.sqrt(M))   # ln(eps*sqrt(M))
RG = [[0, 1], [2, 3], [4, 5], [6, 7]]

_CACHE = {}
KSTAGE = os.environ.get("KSTAGE", "full")


def build_nc():
    nc = bacc.Bacc("TRN2", target_bir_lowering=False, debug=False,
                   num_devices=8)

    def din(name, shape, dtype=f32):
        return nc.dram_tensor(name, list(shape), dtype,
                              kind="ExternalInput").ap()

    xr = din("xr", (128, KD, S2))
    memr = din("memr", (128, KD, S2), bf16)
    wg = {}
    for pre in ("sa", "ca"):
        for nm in ("wq", "wk", "wv", "wo"):
            wg[pre + nm] = din(pre + nm, (128, KD, D), bf16)
        wg[pre + "wf"] = din(pre + "wf", (64, 256), bf16)
        for nm in ("bq", "bk", "bo"):
            wg[pre + nm] = din(pre + nm, (128, KD))
        wg[pre + "bv"] = din(pre + "bv", (1, D))
    w1 = din("w1", (128, KD, F), bf16)
    w2r = din("w2r", (KD, 128, MF, 128), bf16)
    b1_d = din("b1", (128, MF))
    b2_d = din("b2", (128, KD))
    ln_d = {}
    for i in ("1", "2", "3"):
        ln_d["g" + i] = din("ln%s_g" % i, (128, KD))
        ln_d["b" + i] = din("ln%s_b" % i, (128, KD))
    c_invd_d = din("c_invd", (128, 128), bf16)
    c_negh_d = din("c_negh", (128, 2), bf16)
    c_posh_d = din("c_posh", (128, 2), bf16)
    ident_d = din("c_ident", (128, 128), bf16)

    outT = nc.dram_tensor("outT", [128, KD, S2], f32,
                          kind="ExternalOutput").ap()

    with tile.TileContext(nc) as tc:
        with ExitStack() as top:
            dram = top.enter_context(tc.tile_pool(name="dram", bufs=1,
                                                  space="DRAM"))
            ccs = {}
            for pre in ("sa", "ca"):
                ccs[pre + "kv_i"] = dram.tile([65, H, 257], bf16,
                                              name=pre + "kv_i")
                ccs[pre + "kv_o"] = dram.tile([65, H, 257], bf16,
                                              name=pre + "kv_o")
                ccs[pre + "mx_i"] = dram.tile([1, H], f32, name=pre + "mx_i")
                ccs[pre + "mx_o"] = dram.tile([1, H], f32, name=pre + "mx_o")

            const = top.enter_context(tc.tile_pool(name="const", bufs=1))
            c_invd = const.tile([128, 128], bf16, name="c_invd")
            nc.sync.dma_start(out=c_invd[:], in_=c_invd_d[:])
            c_negh = const.tile([128, 2], bf16, name="c_negh")
            nc.sync.dma_start(out=c_negh[:], in_=c_negh_d[:])
            c_posh = const.tile([128, 2], bf16, name="c_posh")
            nc.sync.dma_start(out=c_posh[:], in_=c_posh_d[:])
            ident = const.tile([128, 128], bf16, name="ident")
            nc.sync.dma_start(out=ident[:], in_=ident_d[:])
            ones_bf = const.tile([128, 1], bf16, name="ones_bf")
            nc.vector.memset(ones_bf[:], 1.0)
            c_eps = const.tile([128, 1], bf16, name="c_eps")
            nc.vector.memset(c_eps[:], 1.0e-5)
            c_lna = const.tile([128, 1], f32, name="c_lna")
            nc.vector.memset(c_lna[:], LNA)
            cb = {}
            for pre in ("sa", "ca"):
                for nm in ("bq", "bk", "bo"):
                    t = const.tile([128, KD], f32, name=pre + nm)
                    nc.sync.dma_start(out=t[:], in_=wg[pre + nm][:])
                    cb[pre + nm] = t
                t = const.tile([1, D], f32, name=pre + "bv")
                nc.sync.dma_start(out=t[:], in_=wg[pre + "bv"][:])
                cb[pre + "bv"] = t
                wfl = const.tile([128, 256], bf16, name=pre + "wfl")
                nc.vector.memset(wfl[64:128, :], 0.0)
                nc.sync.dma_start(out=wfl[0:64, :], in_=wg[pre + "wf"][:])
                wfh = const.tile([128, 256], bf16, name=pre + "wfh")
                nc.vector.memset(wfh[0:64, :], 0.0)
                nc.sync.dma_start(out=wfh[64:128, :], in_=wg[pre + "wf"][:])
                cb[pre + "wf"] = (wfl, wfh)
            for nm, d_ap in (("b1", b1_d), ("b2", b2_d)):
                t = const.tile(list(d_ap.shape), f32, name=nm)
                nc.sync.dma_start(out=t[:], in_=d_ap[:])
                cb[nm] = t
            for k, d_ap in ln_d.items():
                t = const.tile([128, KD], f32, name="ln" + k)
                nc.sync.dma_start(out=t[:], in_=d_ap[:])
                cb["ln" + k] = t

            # ------------------------------------------------ helpers
            def dump8(src_ap, dtype=f32):
                """Dump a [128, KD, S2]-like tile to outT."""
                with ExitStack() as ph:
                    sbd = ph.enter_context(tc.tile_pool(name="dmp", bufs=2))
                    for kd in range(KD):
                        dt_ = sbd.tile([128, S2], f32, tag="dmp", name="dmp")
                        nc.vector.tensor_copy(dt_[:], src_ap[:, kd, :])
                        nc.sync.dma_start(out=outT[:, kd, :], in_=dt_[:])

            def ln_stage(x_t, g_t, b_t, out_t, ctx):
                """x_t [128, KD, S2] bf16 -> out_t normalized bf16."""
                ps = ctx.enter_context(tc.tile_pool(name="lnps", bufs=2,
                                                    space="PSUM"))
                sb = ctx.enter_context(tc.tile_pool(name="lnsb", bufs=2))
                for ch in range(NCH):
                    cs = bass.ts(ch, 512)
                    mv = ps.tile([128, 2, 512], f32, tag="ln_ps", name="ln_ps")
                    for kd in range(KD):
                        nc.tensor.matmul(mv[:, 0, :], c_invd[:], x_t[:, kd, cs],
                                         start=(kd == 0), stop=(kd == KD - 1),
                                         skip_group_check=True)
                    for kd in range(KD):
                        x2 = sb.tile([128, 512], bf16, tag="ln_x2", name="ln_x2")
                        nc.vector.tensor_mul(x2[:], x_t[:, kd, cs], x_t[:, kd, cs])
                        nc.tensor.matmul(mv[:, 1, :], c_invd[:], x2[:],
                                         start=(kd == 0), stop=(kd == KD - 1),
                                         skip_group_check=True)
                    mu = sb.tile([128, 512], bf16, tag="ln_mu", name="ln_mu")
                    nc.scalar.copy(mu[:], mv[:, 0, :])
                    mu2 = sb.tile([128, 512], bf16, tag="ln_mu2", name="ln_mu2")
                    nc.vector.tensor_mul(mu2[:], mu[:], mu[:])
                    m2 = sb.tile([128, 512], bf16, tag="ln_m2", name="ln_m2")
                    nc.scalar.copy(m2[:], mv[:, 1, :])
                    var = sb.tile([128, 512], bf16, tag="ln_var", name="ln_var")
                    nc.vector.tensor_sub(var[:], m2[:], mu2[:])
                    sd = sb.tile([128, 512], bf16, tag="ln_sd", name="ln_sd")
                    nc.scalar.activation(sd[:], var[:], AF.Sqrt, bias=c_eps[:])
                    rstd = sb.tile([128, 512], f32, tag="ln_rstd",
                                   name="ln_rstd")
                    nc.vector.reciprocal(rstd[:], sd[:])
                    for kd in range(KD):
                        xm = sb.tile([128, 512], bf16, tag="ln_xm", name="ln_xm")
                        nc.vector.tensor_sub(xm[:], x_t[:, kd, cs], mu[:])
                        xn = sb.tile([128, 512], bf16, tag="ln_xn", name="ln_xn")
                        nc.vector.tensor_mul(xn[:], xm[:], rstd[:])
                        nc.vector.tensor_scalar(out_t[:, kd, cs], xn[:],
                                                g_t[:, kd:kd + 1],
                                                b_t[:, kd:kd + 1],
                                                ALU.mult, ALU.add)

            def proj_feat(src_t, w_sb, b_t, out_t, ctx, ps):
                """out_t [128, KD, S2] bf16 (feature layout) = W @ src."""
                for mt in range(KD):
                    for ch in range(NCH):
                        cs = bass.ts(ch, 512)
                        o_ps = ps.tile([128, 512], f32, tag="pj_ps",
                                       name="pj_ps")
                        for kd in range(KD):
                            nc.tensor.matmul(
                                o_ps[:], w_sb[:, kd, mt * 128:(mt + 1) * 128],
                                src_t[:, kd, cs], start=(kd == 0),
                                stop=(kd == KD - 1), skip_group_check=True)
                        nc.scalar.activation(out_t[:, mt, cs], o_ps[:],
                                             AF.Identity,
                                             bias=b_t[:, mt:mt + 1])

            def proj_v(src_t, wv_sb, bv_t, v_aug, ctx, ps, sb):
                """v_aug [128, RT, H, 65] bf16 (row layout), col 64 = 1."""
                bv_rep = sb.tile([128, D], f32, tag="bv_rep", name="bv_rep")
                nc.gpsimd.partition_broadcast(bv_rep[:], bv_t[:], channels=128)
                nc.vector.memset(v_aug[:, :, :, 64:65], 1.0)
                for rt in range(RT):
                    for c2 in range(2):
                        v_ps = ps.tile([128, 512], f32, tag="v_ps",
                                       name="v_ps")
                        for kd in range(KD):
                            nc.tensor.matmul(
                                v_ps[:], src_t[:, kd, rt * 128:(rt + 1) * 128],
                                wv_sb[:, kd, c2 * 512:(c2 + 1) * 512],
                                start=(kd == 0), stop=(kd == KD - 1),
                                skip_group_check=True)
                        nc.vector.tensor_add(
                            v_aug[:, rt, c2 * 8:(c2 + 1) * 8, 0:64], v_ps[:],
                            bv_rep[:, c2 * 512:(c2 + 1) * 512])

            def phi_k_kv(kT, v_aug, wf_pair, kv_sb, stab_all, ctx):
                """E_k per pair; kv/ksum/vsum into kv_sb [65, H, 257] bf16;
                per-head local stab max into stab_all [1, H] f32."""
                ps_pj = ctx.enter_context(tc.tile_pool(name="ps_pj", bufs=3,
                                                       space="PSUM"))
                ps_ns = ctx.enter_context(tc.tile_pool(name="ps_ns", bufs=2,
                                                       space="PSUM"))
                ps_kv = ctx.enter_context(tc.tile_pool(name="ps_kv", bufs=2,
                                                       space="PSUM"))
                sb = ctx.enter_context(tc.tile_pool(name="pk_sb", bufs=3))
                ek_p = ctx.enter_context(tc.tile_pool(name="ek", bufs=2))
                for p in range(NP):
                    k2 = sb.tile([128, S2], bf16, tag="k2", name="k2")
                    nc.vector.tensor_mul(k2[:], kT[:, p, :], kT[:, p, :])
                    E_t = ek_p.tile([128, RT, 2, 257], bf16, tag="E_t",
                                    name="E_t")
                    nc.vector.memset(E_t[:, :, :, 256:257], 1.0)
                    stab_run = sb.tile([128, 2], f32, tag="st_run",
                                       name="st_run")
                    for rt in range(RT):
                        rs = bass.ts(rt, 128)
                        pj = ps_pj.tile([128, 2, 256], f32, tag="pj",
                                        name="pj_k")
                        for h in range(2):
                            nc.tensor.matmul(pj[:, h, :], kT[:, p, rs],
                                             wf_pair[h][:])
                        nsq = ps_ns.tile([128, 2], f32, tag="nsq", name="nsq")
                        nc.tensor.matmul(nsq[:], k2[:, rs], c_negh[:])
                        rmax = sb.tile([128, 2], f32, tag="rmax", name="rmax")
                        nc.vector.reduce_max(rmax[:], pj[:], axis=AX.X)
                        if rt == 0:
                            nc.vector.tensor_copy(stab_run[:], rmax[:])
                        else:
                            nc.vector.tensor_max(stab_run[:], stab_run[:],
                                                 rmax[:])
                        nsq_sb = sb.tile([128, 2], f32, tag="nsq_sb",
                                         name="nsq_sb")
                        nc.vector.tensor_copy(nsq_sb[:], nsq[:])
                        for h in range(2):
                            nc.scalar.activation(E_t[:, rt, h, 0:256],
                                                 pj[:, h, :], AF.Exp,
                                                 bias=nsq_sb[:, h:h + 1])
                    stab_rep = sb.tile([128, 2], f32, tag="st_rep",
                                       name="st_rep")
                    nc.gpsimd.partition_all_reduce(
                        stab_rep[:], stab_run[:], channels=128,
                        reduce_op=bass_isa.ReduceOp.max)
                    nc.vector.tensor_copy(stab_all[0:1, 2 * p:2 * p + 2],
                                          stab_rep[0:1, :])
                    for h in range(2):
                        kv_ps = ps_kv.tile([65, 257], f32, tag="kv_ps",
                                           name="kv_ps")
                        for rt in range(RT):
                            nc.tensor.matmul(kv_ps[:],
                                             v_aug[:, rt, 2 * p + h, :],
                                             E_t[:, rt, h, :], start=(rt == 0),
                                             stop=(rt == RT - 1),
                                             skip_group_check=True)
                        nc.vector.tensor_copy(kv_sb[:, 2 * p + h, :], kv_ps[:])

            def phi_q(qT, wf_pair, Eq, gam, ctx):
                """Eq [128, NP, 2, 2, S2] bf16 = exp(projq) in [m,s] layout;
                gam [1, NP, 2, S2] bf16 = eps*sqrt(M)*exp(sq)*rowsum."""
                ps_pj = ctx.enter_context(tc.tile_pool(name="psq", bufs=4,
                                                       space="PSUM"))
                ps_sq = ctx.enter_context(tc.tile_pool(name="pssq", bufs=2,
                                                       space="PSUM"))
                ps_rs = ctx.enter_context(tc.tile_pool(name="psrs", bufs=2,
                                                       space="PSUM"))
                sb = ctx.enter_context(tc.tile_pool(name="pq_sb", bufs=3))
                for p in range(NP):
                    for ch in range(NCH):
                        cs = bass.ts(ch, 512)
                        q2 = sb.tile([128, 512], bf16, tag="q2", name="q2")
                        nc.vector.tensor_mul(q2[:], qT[:, p, cs], qT[:, p, cs])
                        for h in range(2):
                            for mt in range(2):
                                pj = ps_pj.tile([128, 512], f32, tag="pjq",
                                                name="pjq")
                                nc.tensor.matmul(
                                    pj[:],
                                    wf_pair[h][:, mt * 128:(mt + 1) * 128],
                                    qT[:, p, cs])
                                nc.scalar.activation(Eq[:, p, h, mt, cs],
                                                     pj[:], AF.Exp)
                        for h in range(2):
                            sq_ps = ps_sq.tile([1, 512], f32, tag="sq",
                                               name="sq")
                            nc.tensor.matmul(sq_ps[:], c_posh[:, h:h + 1],
                                             q2[:])
                            esq = sb.tile([1, 512], f32, tag="esq", name="esq")
                            nc.scalar.activation(esq[:], sq_ps[:], AF.Exp,
                                                 bias=c_lna[0:1, :])
                            rs_ps = ps_rs.tile([1, 512], f32, tag="rs",
                                               name="rs")
                            for mt in range(2):
                                nc.tensor.matmul(rs_ps[:], ones_bf[:],
                                                 Eq[:, p, h, mt, cs],
                                                 start=(mt == 0),
                                                 stop=(mt == 1),
                                                 skip_group_check=True)
                            nc.vector.tensor_mul(gam[0:1, p, h, cs], esq[:],
                                                 rs_ps[:])

            def kv_finish(kv_cc, mx_cc, kvT, ck_row, ctx):
                """Post-AR: beta-correct kv, transpose to kvT [128,H,2,65],
                colkv rows ck_row [1, H, 65]."""
                sb = ctx.enter_context(tc.tile_pool(name="kf_sb", bufs=2))
                ps_tp = ctx.enter_context(tc.tile_pool(name="ps_tp", bufs=2,
                                                       space="PSUM"))
                ps_ck = ctx.enter_context(tc.tile_pool(name="ps_ck", bufs=2,
                                                       space="PSUM"))
                kv_r = sb.tile([65, H, 257], bf16, tag="kv_r", name="kv_r")
                nc.sync.dma_start(out=kv_r[:], in_=kv_cc[:])
                stab_g = sb.tile([1, H], f32, tag="stab_g", name="stab_g")
                nc.sync.dma_start(out=stab_g[:], in_=mx_cc[:])
                betae = sb.tile([1, H], f32, tag="betae", name="betae")
                nc.scalar.activation(betae[:], stab_g[:], AF.Exp,
                                     bias=c_lna[0:1, :])
                beta_bc = sb.tile([128, H], f32, tag="beta_bc", name="beta_bc")
                nc.gpsimd.partition_broadcast(beta_bc[:], betae[:],
                                              channels=128)
                for h in range(H):
                    vsb = sb.tile([65, 1], f32, tag="vsb", name="vsb")
                    nc.vector.tensor_mul(vsb[:], kv_r[:, h, 256:257],
                                         beta_bc[0:65, h:h + 1])
                    kvc = sb.tile([65, 256], bf16, tag="kvc", name="kvc")
                    nc.vector.tensor_scalar(kvc[:], kv_r[:, h, 0:256], vsb[:],
                                            None, ALU.add)
                    for mt in range(2):
                        tp = ps_tp.tile([128, 65], bf16, tag="tp", name="tp")
                        nc.tensor.transpose(tp[:],
                                            kvc[0:65, mt * 128:(mt + 1) * 128],
                                            ident[0:65, 0:65])
                        nc.vector.tensor_copy(kvT[:, h, mt, :], tp[:])
                    ck_ps = ps_ck.tile([1, 65], f32, tag="ck", name="ck")
                    for mt in range(2):
                        nc.tensor.matmul(ck_ps[:], ones_bf[:],
                                         kvT[:, h, mt, :], start=(mt == 0),
                                         stop=(mt == 1),
                                         skip_group_check=True)
                    nc.vector.tensor_copy(ck_row[0:1, h, :], ck_ps[:])

            def attn_out(Eq, gam, kvT, ck_row, attn_t, ctx):
                """attn_t [128, NP, S2] bf16 feature layout."""
                ps_o = ctx.enter_context(tc.tile_pool(name="ps_o", bufs=3,
                                                      space="PSUM"))
                sb = ctx.enter_context(tc.tile_pool(name="ao_sb", bufs=3))
                for p in range(NP):
                    for h in range(2):
                        hd = 2 * p + h
                        hp = slice(64 * h, 64 * h + 64)
                        for ch in range(NCH):
                            cs = bass.ts(ch, 512)
                            o_ps = ps_o.tile([65, 512], f32, tag="o_ps",
                                             name="o_ps")
                            for mt in range(2):
                                nc.tensor.matmul(o_ps[:], kvT[:, hd, mt, :],
                                                 Eq[:, p, h, mt, cs],
                                                 start=(mt == 0), stop=False,
                                                 skip_group_check=True)
                            nc.tensor.matmul(o_ps[:], ck_row[0:1, hd, :],
                                             gam[0:1, p, h, cs], start=False,
                                             stop=True, skip_group_check=True)
                            zr = sb.tile([1, 512], f32, tag="zr", name="zr")
                            nc.vector.reciprocal(zr[:], o_ps[64:65, :])
                            zb = sb.tile([64, 512], f32, tag="zb", name="zb")
                            nc.gpsimd.partition_broadcast(zb[:], zr[:],
                                                          channels=64)
                            nc.vector.tensor_mul(attn_t[hp, p, cs],
                                                 o_ps[0:64, :], zb[:])

            def wo_resid(attn_t, wo_sb, bo_t, x_t, x_new, ctx, to_dram=None):
                """x_new = x_t + Wo @ attn + bo; optionally stream to DRAM."""
                ps = ctx.enter_context(tc.tile_pool(name="wops", bufs=3,
                                                    space="PSUM"))
                sb = ctx.enter_context(tc.tile_pool(name="wosb", bufs=3))
                for md in range(KD):
                    for ch in range(NCH):
                        cs = bass.ts(ch, 512)
                        o_ps = ps.tile([128, 512], f32, tag="wo_ps",
                                       name="wo_ps")
                        for kp in range(KD):
                            nc.tensor.matmul(o_ps[:],
                                             wo_sb[:, kp, md * 128:(md + 1) * 128],
                                             attn_t[:, kp, cs],
                                             start=(kp == 0),
                                             stop=(kp == KD - 1),
                                             skip_group_check=True)
                        ev = sb.tile([128, 512], bf16, tag="wo_ev",
                                     name="wo_ev")
                        nc.scalar.activation(ev[:], o_ps[:], AF.Identity,
                                             bias=bo_t[:, md:md + 1])
                        nc.vector.tensor_add(x_new[:, md, cs], x_t[:, md, cs],
                                             ev[:])

            # ================================================= program
            xp0 = ExitStack()
            xp1 = ExitStack()
            xp2 = ExitStack()
            x0 = xp0.enter_context(tc.tile_pool(name="xp0", bufs=1,
                                                 side="right")).tile(
                [128, KD, S2], bf16, name="x0")
            x1 = xp1.enter_context(tc.tile_pool(name="xp1", bufs=1)).tile(
                [128, KD, S2], bf16, name="x1")

            with ExitStack() as lx:
                sb = lx.enter_context(tc.tile_pool(name="lx_sb", bufs=2))
                for kd in range(KD):
                    xa = sb.tile([128, S2], f32, tag="xa", name="xa")
                    nc.sync.dma_start(out=xa[:], in_=xr[:, kd, :])
                    nc.vector.tensor_copy(x0[:, kd, :], xa[:])

            kvp_sa = ExitStack()
            kvp_ca = ExitStack()
            _kp_sa = kvp_sa.enter_context(tc.tile_pool(name="kvpsa", bufs=1,
                                                       side="right"))
            kvT_sa = _kp_sa.tile([128, H, 2, 65], bf16, name="kvT_sa")
            ck_sa = _kp_sa.tile([1, H, 65], bf16, name="ck_sa")

            # ---------- CA k/v from memory; fire its ARs early
            with ExitStack() as cakv:
                wpool = cakv.enter_context(tc.tile_pool(name="cakw", bufs=1))
                ca_wk = wpool.tile([128, KD, D], bf16, name="ca_wk")
                nc.sync.dma_start(out=ca_wk[:], in_=wg["cawk"][:])
                ca_wv = wpool.tile([128, KD, D], bf16, name="ca_wv")
                nc.sync.dma_start(out=ca_wv[:], in_=wg["cawv"][:])
                mem_p = cakv.enter_context(tc.tile_pool(name="memp", bufs=1))
                mem = mem_p.tile([128, KD, S2], bf16, name="mem")
                nc.gpsimd.dma_start(out=mem[:], in_=memr[:])
                v_ca_p = cakv.enter_context(tc.tile_pool(name="vca", bufs=1))
                v_aug = v_ca_p.tile([128, RT, H, 65], bf16, name="v_aug_ca")
                kT_p = cakv.enter_context(tc.tile_pool(name="ktca", bufs=1))
                kT = kT_p.tile([128, NP, S2], bf16, name="kT_ca")
                sb = cakv.enter_context(tc.tile_pool(name="cas", bufs=2))
                with ExitStack() as prjs:
                    ps = prjs.enter_context(tc.tile_pool(name="cap", bufs=3,
                                                         space="PSUM"))
                    proj_feat(mem, ca_wk, cb["cabk"], kT, cakv, ps)
                    proj_v(mem, ca_wv, cb["cabv"], v_aug, cakv, ps, sb)
                kvsb_p = cakv.enter_context(tc.tile_pool(name="cakvsb",
                                                          bufs=1))
                kv_sb = kvsb_p.tile([65, H, 257], bf16, name="kv_sb")
                stab_all = kvsb_p.tile([1, H], f32, name="stab_all")
                with ExitStack() as ph:
                    phi_k_kv(kT, v_aug, cb["cawf"], kv_sb, stab_all, ph)
                nc.sync.dma_start(out=ccs["camx_i"][:], in_=stab_all[:])
                nc.gpsimd.collective_compute(
                    "AllReduce", ALU.max, replica_groups=RG,
                    ins=[ccs["camx_i"].opt()], outs=[ccs["camx_o"].opt()])
                nc.sync.dma_start(out=ccs["cakv_i"][:], in_=kv_sb[:])
                nc.gpsimd.collective_compute(
                    "AllReduce", ALU.add, replica_groups=RG,
                    ins=[ccs["cakv_i"].opt()], outs=[ccs["cakv_o"].opt()])

            # ---------- LN1 + SA
            def favor_local(pre, x_in, x_out, kvT_t, ck_t, lng, lnb,
                            kv_from=None):
                """One attention block: LN -> proj -> phi -> out -> Wo."""
                blk = ExitStack()
                try:
                    _ = None
                    t2s = ExitStack()
                    t2p = t2s.enter_context(tc.tile_pool(name=pre + "t2",
                                                         bufs=1))
                    t2 = t2p.tile([128, KD, S2], bf16, name=pre + "t2")
                    with ExitStack() as lnc_:
                        ln_stage(x_in, lng, lnb, t2, lnc_)
                    if KSTAGE == "ln" and pre == "sa":
                        dump8(t2)
                        return
                    wq_s = ExitStack()
                    wq_p = wq_s.enter_context(tc.tile_pool(name=pre + "wqp",
                                                           bufs=1))
                    wq_sb = wq_p.tile([128, KD, D], bf16, name=pre + "wq_sb")
                    nc.sync.dma_start(out=wq_sb[:], in_=wg[pre + "wq"][:])
                    qT_s = ExitStack()
                    qT_p = qT_s.enter_context(tc.tile_pool(name=pre + "qtp",
                                                           bufs=1,
                                                           side="right"))
                    qT = qT_p.tile([128, NP, S2], bf16, name=pre + "qT")
                    if pre == "sa":
                        # SA: k/v/phi_k/kv from t2, then AR; q work overlaps AR
                        with ExitStack() as sakv:
                            wpool = sakv.enter_context(
                                tc.tile_pool(name="sakw", bufs=1))
                            wk_sb = wpool.tile([128, KD, D], bf16,
                                               name="sa_wk_sb")
                            nc.sync.dma_start(out=wk_sb[:], in_=wg["sawk"][:])
                            wv_sb = wpool.tile([128, KD, D], bf16,
                                               name="sa_wv_sb")
                            nc.sync.dma_start(out=wv_sb[:], in_=wg["sawv"][:])
                            v_p = sakv.enter_context(
                                tc.tile_pool(name="vsa", bufs=1))
                            v_aug = v_p.tile([128, RT, H, 65], bf16,
                                             name="v_aug_sa")
                            kT_p2 = sakv.enter_context(
                                tc.tile_pool(name="ktsa", bufs=1))
                            kT = kT_p2.tile([128, NP, S2], bf16, name="kT_sa")
                            sb = sakv.enter_context(
                                tc.tile_pool(name="sas", bufs=2))
                            with ExitStack() as prjs:
                                ps = prjs.enter_context(
                                    tc.tile_pool(name="sap", bufs=3,
                                                 space="PSUM"))
                                proj_feat(t2, wk_sb, cb["sabk"], kT, sakv, ps)
                                proj_v(t2, wv_sb, cb["sabv"], v_aug, sakv, ps,
                                       sb)
                            kvsb_p = sakv.enter_context(
                                tc.tile_pool(name="sakvsb", bufs=1))
                            kv_sb = kvsb_p.tile([65, H, 257], bf16,
                                                name="kv_sb_sa")
                            stab_all = kvsb_p.tile([1, H], f32,
                                                   name="stab_all_sa")
                            with ExitStack() as ph:
                                phi_k_kv(kT, v_aug, cb["sawf"], kv_sb,
                                         stab_all, ph)
                            nc.sync.dma_start(out=ccs["samx_i"][:],
                                              in_=stab_all[:])
                            nc.gpsimd.collective_compute(
                                "AllReduce", ALU.max, replica_groups=RG,
                                ins=[ccs["samx_i"].opt()],
                                outs=[ccs["samx_o"].opt()])
                            nc.sync.dma_start(out=ccs["sakv_i"][:],
                                              in_=kv_sb[:])
                            nc.gpsimd.collective_compute(
                                "AllReduce", ALU.add, replica_groups=RG,
                                ins=[ccs["sakv_i"].opt()],
                                outs=[ccs["sakv_o"].opt()])
                            # q projection after the ARs fire: overlaps them
                            with ExitStack() as prjq:
                                psq = prjq.enter_context(
                                    tc.tile_pool(name="sapq", bufs=3,
                                                 space="PSUM"))
                                proj_feat(t2, wq_sb, cb["sabq"], qT, sakv,
                                          psq)
                            if KSTAGE == "qk":
                                dump8(qT)
                                return
                    else:
                        # CA: q projection only (k/v done early)
                        with ExitStack() as caq:
                            ps = caq.enter_context(
                                tc.tile_pool(name="cap2", bufs=3,
                                             space="PSUM"))
                            proj_feat(t2, wq_sb, cb["cabq"], qT, caq, ps)
                    wq_s.close()
                    t2s.close()
                    Eq_s = ExitStack()
                    Eq_p = Eq_s.enter_context(tc.tile_pool(name=pre + "eqp",
                                                           bufs=1))
                    Eq = Eq_p.tile([128, NP, 2, 2, S2], bf16, name=pre + "Eq")
                    gam = Eq_p.tile([1, NP, 2, S2], bf16, name=pre + "gam")
                    with ExitStack() as ph:
                        phi_q(qT, cb[pre + "wf"], Eq, gam, ph)
                    qT_s.close()
                    attn_s = ExitStack()
                    attn_p = attn_s.enter_context(
                        tc.tile_pool(name=pre + "atp", bufs=1, side="right"))
                    attn_t = attn_p.tile([128, NP, S2], bf16,
                                         name=pre + "attn")
                    with ExitStack() as ph:
                        kv_finish(ccs[pre + "kv_o"], ccs[pre + "mx_o"],
                                  kvT_t, ck_t, ph)
                    if KSTAGE == "kv" and pre == "sa":
                        with ExitStack() as ph:
                            sbd = ph.enter_context(tc.tile_pool(name="dmp2",
                                                                bufs=2))
                            for hh in range(KD):
                                dt_ = sbd.tile([128, S2], f32, tag="dkv",
                                               name="dkv")
                                nc.vector.memset(dt_[:], 0.0)
                                # heads 2*hh, 2*hh+1 -> 4 x 65-col blocks
                                for j in range(2):
                                    for mt in range(2):
                                        nc.vector.tensor_copy(
                                            dt_[:, (2 * j + mt) * 65:
                                                (2 * j + mt) * 65 + 65],
                                            kvT_t[:, 2 * hh + j, mt, :])
                                nc.sync.dma_start(out=outT[:, hh, :],
                                                  in_=dt_[:])
                        return
                    wo_p = ExitStack()
                    wo_pool = wo_p.enter_context(
                        tc.tile_pool(name=pre + "wop", bufs=1, side="right"))
                    wo_sb = wo_pool.tile([128, KD, D], bf16,
                                         name=pre + "wo_sb")
                    nc.sync.dma_start(out=wo_sb[:], in_=wg[pre + "wo"][:])
                    with ExitStack() as ph:
                        attn_out(Eq, gam, kvT_t, ck_t, attn_t, ph)
                    Eq_s.close()
                    if KSTAGE == "attn" and pre == "sa":
                        dump8(attn_t)
                        return
                    with ExitStack() as ph:
                        wo_resid(attn_t, wo_sb, cb[pre + "bo"], x_in, x_out,
                                 ph)
                    wo_p.close()
                finally:
                    for _st in ("wo_p", "attn_s", "Eq_s", "qT_s", "wq_s",
                                "t2s"):
                        _obj = locals().get(_st)
                        if _obj is not None:
                            _obj.close()
                    blk.close()

            favor_local("sa", x0, x1, kvT_sa, ck_sa, cb["lng1"], cb["lnb1"])
            done = KSTAGE in ("ln", "qk", "kv", "attn")
            if KSTAGE == "sa":
                dump8(x1)
                done = True
            if not done:
                kvp_sa.close()
                xp0.close()
                x2 = xp2.enter_context(tc.tile_pool(name="xp2", bufs=1,
                                                    side="right")).tile(
                    [128, KD, S2], bf16, name="x2")
                _kp_ca = kvp_ca.enter_context(tc.tile_pool(name="kvpca",
                                                           bufs=1,
                                                           side="right"))
                kvT_ca = _kp_ca.tile([128, H, 2, 65], bf16, name="kvT_ca")
                ck_ca = _kp_ca.tile([1, H, 65], bf16, name="ck_ca")
                favor_local("ca", x1, x2, kvT_ca, ck_ca, cb["lng2"],
                            cb["lnb2"])
                if KSTAGE == "ca":
                    dump8(x2)
                    done = True
                else:
                    xp1.close()
                    kvp_ca.close()

            # ---------- LN3 + FFN
            if not done:
                with ExitStack() as ffn:
                    ft2_s = ExitStack()
                    t2p = ft2_s.enter_context(tc.tile_pool(name="ff_t2",
                                                           bufs=1))
                    t2b = t2p.tile([128, KD, S2], bf16, name="t2b")
                    with ExitStack() as lnc_:
                        ln_stage(x2, cb["lng3"], cb["lnb3"], t2b, lnc_)
                    w1_s = ExitStack()
                    w1_p = w1_s.enter_context(tc.tile_pool(name="w1p",
                                                           bufs=1))
                    w1_sb = w1_p.tile([128, KD, F], bf16, name="w1_sb")
                    nc.sync.dma_start(out=w1_sb[:], in_=w1[:])
                    h_p = ffn.enter_context(tc.tile_pool(name="hp", bufs=1,
                                                         side="right"))
                    h1 = h_p.tile([128, MF, S2], bf16, name="h1")
                    ps1 = ffn.enter_context(tc.tile_pool(name="f1ps", bufs=4,
                                                         space="PSUM"))
                    for mf in range(MF):
                        for ch in range(NCH):
                            cs = bass.ts(ch, 512)
                            o_ps = ps1.tile([128, 512], f32, tag="f1",
                                            name="f1")
                            for kd in range(KD):
                                nc.tensor.matmul(
                                    o_ps[:],
                                    w1_sb[:, kd, mf * 128:(mf + 1) * 128],
                                    t2b[:, kd, cs], start=(kd == 0),
                                    stop=(kd == KD - 1),
                                    skip_group_check=True)
                            nc.scalar.activation(h1[:, mf, cs], o_ps[:],
                                                 AF.Relu,
                                                 bias=cb["b1"][:, mf:mf + 1])
                    w1_s.close()
                    ft2_s.close()
                    w2_pool = ffn.enter_context(tc.tile_pool(name="w2p",
                                                             bufs=2))
                    ps2 = ffn.enter_context(tc.tile_pool(name="f2ps", bufs=4,
                                                         space="PSUM"))
                    sb = ffn.enter_context(tc.tile_pool(name="ffsb", bufs=3))
                    for md in range(KD):
                        w2_t = w2_pool.tile([128, MF, 128], bf16, tag="w2t",
                                            name="w2t")
                        nc.sync.dma_start(out=w2_t[:], in_=w2r[md, :, :, :])
                        for ch in range(NCH):
                            cs = bass.ts(ch, 512)
                            o_ps = ps2.tile([128, 512], f32, tag="f2",
                                            name="f2")
                            for kf in range(MF):
                                nc.tensor.matmul(o_ps[:], w2_t[:, kf, :],
                                                 h1[:, kf, cs],
                                                 start=(kf == 0),
                                                 stop=(kf == MF - 1),
                                                 skip_group_check=True)
                            ev = sb.tile([128, 512], f32, tag="f2e",
                                         name="f2e")
                            nc.scalar.activation(ev[:], o_ps[:], AF.Identity,
                                                 bias=cb["b2"][:, md:md + 1])
                            xo = sb.tile([128, 512], f32, tag="xo", name="xo")
                            nc.vector.tensor_add(xo[:], x2[:, md, cs], ev[:])
                            nc.sync.dma_start(out=outT[:, md, cs], in_=xo[:])
            for _st in (kvp_ca, xp2, kvp_sa, xp0, xp1):
                _st.close()
    nc.finalize()
    return nc


# ------------------------------------------------------------------ host

def _prep_inputs(inputs):
    Cs = DH ** -0.25
    f = np.float32
    bfd = ml_dtypes.bfloat16
    inp = {k: np.asarray(v, dtype=f) for k, v in inputs.items()}

    def fshape(vec, dt=f):
        n = vec.shape[0] // 128
        return np.ascontiguousarray(vec.reshape(n, 128).T).astype(dt)

    def wblock(wT):
        # [D_in, N] -> [128, D_in//128, N]
        n = wT.shape[0] // 128
        return np.ascontiguousarray(
            wT.reshape(n, 128, wT.shape[1]).transpose(1, 0, 2))

    consts = {}
    consts["c_invd"] = np.full((128, 128), 1.0 / D, bfd)
    blk = np.zeros((128, 2), f)
    blk[0:64, 0] = -0.5
    blk[64:128, 1] = -0.5
    consts["c_negh"] = blk.astype(bfd)
    consts["c_posh"] = (-blk).astype(bfd)
    consts["c_ident"] = np.eye(128, dtype=bfd)

    shared = dict(consts)
    for pre in ("sa", "ca"):
        shared[pre + "wq"] = wblock(inp[pre + "_wq"].T * Cs).astype(bfd)
        shared[pre + "wk"] = wblock(inp[pre + "_wk"].T * Cs).astype(bfd)
        shared[pre + "wv"] = wblock(inp[pre + "_wv"].T).astype(bfd)
        shared[pre + "wo"] = wblock(inp[pre + "_wo"].T).astype(bfd)
        shared[pre + "wf"] = np.ascontiguousarray(
            inp[pre + "_feat"].T).astype(bfd)
        shared[pre + "bq"] = fshape(inp[pre + "_bq"] * Cs)
        shared[pre + "bk"] = fshape(inp[pre + "_bk"] * Cs)
        shared[pre + "bv"] = inp[pre + "_bv"].reshape(1, D).copy()
        shared[pre + "bo"] = fshape(inp[pre + "_bo"])
    shared["w1"] = wblock(inp["ff_w1"].T).astype(bfd)
    w2T = inp["ff_w2"].T  # [F, D]
    shared["w2r"] = np.ascontiguousarray(
        w2T.reshape(MF, 128, KD, 128).transpose(2, 1, 0, 3)).astype(bfd)
    shared["b1"] = fshape(inp["ff_b1"])
    shared["b2"] = fshape(inp["ff_b2"])
    for i in ("1", "2", "3"):
        shared["ln%s_g" % i] = fshape(inp["ln%s_g" % i])
        shared["ln%s_b" % i] = fshape(inp["ln%s_b" % i])

    in_maps = []
    for core in range(8):
        b, half = core // 2, core % 2
        sl = slice(half * S2, (half + 1) * S2)
        m = dict(shared)
        m["xr"] = wblock(np.ascontiguousarray(inp["tgt"][sl, b, :].T))
        m["memr"] = wblock(np.ascontiguousarray(
            inp["memory"][sl, b, :].T)).astype(bfd)
        in_maps.append(m)
    return in_maps


def _build_exec(nc, n_cores=8):
    import jax
    from jax.sharding import Mesh, PartitionSpec
    from jax.experimental.shard_map import shard_map
    from concourse import bass2jax as b2j

    b2j.install_neuronx_cc_hook()
    partition_name = (nc.partition_id_tensor.name
                      if nc.partition_id_tensor else None)
    in_names, out_names, out_avals = [], [], []
    for alloc in nc.m.functions[0].allocations:
        if not isinstance(alloc, mybir.MemoryLocationSet):
            continue
        name = alloc.memorylocations[0].name
        if alloc.kind == "ExternalInput":
            if name != partition_name:
                in_names.append(name)
        elif alloc.kind == "ExternalOutput":
            out_names.append(name)
            out_avals.append(jax.core.ShapedArray(
                tuple(alloc.tensor_shape), mybir.dt.np(alloc.dtype)))
    n_params = len(in_names)
    all_in = list(in_names) + list(out_names)
    if partition_name is not None:
        all_in.append(partition_name)

    def _body(*args):
        operands = list(args)
        if partition_name is not None:
            operands.append(b2j.partition_id_tensor())
        outs = b2j._bass_exec_p.bind(
            *operands, out_avals=tuple(out_avals), in_names=tuple(all_in),
            out_names=tuple(out_names), lowering_input_output_aliases=(),
            sim_require_finite=True, sim_require_nnan=True, nc=nc)
        return tuple(outs)

    devices = jax.devices()[:n_cores]
    mesh = Mesh(np.asarray(devices), ("core",))
    n_outs = len(out_names)
    specs = (PartitionSpec("core"),) * (n_params + n_outs)
    out_specs = (PartitionSpec("core"),) * n_outs
    donate = tuple(range(n_params, n_params + n_outs))
    sharded = jax.jit(shard_map(_body, mesh=mesh, in_specs=specs,
                                out_specs=out_specs, check_rep=False),
                      donate_argnums=donate, keep_unused=True)

    def run(in_maps, fetch=True):
        import jax as _jax
        concat = [np.concatenate([np.asarray(in_maps[c][nm])
                                  for c in range(n_cores)], axis=0)
                  for nm in in_names]
        zeros = [np.zeros((n_cores * av.shape[0], *av.shape[1:]), av.dtype)
                 for av in out_avals]
        outs = sharded(*concat, *zeros)
        if not fetch:
            _jax.block_until_ready(outs)
            return None
        return [{nm: np.asarray(outs[i]).reshape(
            n_cores, *out_avals[i].shape)[c]
            for i, nm in enumerate(out_names)} for c in range(n_cores)]

    def time_exec(in_maps, iters=8):
        import time as _time
        import jax as _jax
        from jax.sharding import NamedSharding
        sh = NamedSharding(mesh, PartitionSpec("core"))
        concat = [np.concatenate([np.asarray(in_maps[c][nm])
                                  for c in range(n_cores)], axis=0)
                  for nm in in_names]
        dev_in = _jax.device_put(concat, [sh] * len(concat))
        _jax.block_until_ready(dev_in)
        zeros = [np.zeros((n_cores * av.shape[0], *av.shape[1:]), av.dtype)
                 for av in out_avals]
        times = []
        for _ in range(iters):
            zd = _jax.device_put(zeros, [sh] * len(zeros))
            _jax.block_until_ready(zd)
            t0 = _time.time()
            outs = sharded(*dev_in, *zd)
            _jax.block_until_ready(outs)
            times.append(_time.time() - t0)
        return times

    run.in_names = in_names
    run.time_exec = time_exec
    return run


def _get_exec():
    if "exec" not in _CACHE:
        nc = build_nc()
        _CACHE["exec"] = _build_exec(nc)
    return _CACHE["exec"]


def _unblock(o):
    # [128, KD, S2] -> [D, S2]
    return np.ascontiguousarray(o.transpose(1, 0, 2).reshape(D, S2))


def kernel(**inputs):
    run = _get_exec()
    in_maps = _prep_inputs(inputs)
    res = run(in_maps)
    out = np.empty((S, B, D), np.float32)
    for b in range(B):
        for half in range(2):
            o = _unblock(res[2 * b + half]["outT"])
            out[half * S2:(half + 1) * S2, b, :] = o.T
    return out
